# revision 15
# baseline (speedup 1.0000x reference)
"""MultiHeadImageAttentionBlock on 8 Trainium2 NeuronCores (Bass/Tile).

Sharding: core c = 2*b + g handles batch b (of 4) and head-group g (4 of 8
heads). Per core:
  - Q/K/V 3x3 convs as 9 shifted matmuls on padded 34x34 images (f32r, full
    PE rate), K/V with 2-image row-packing (images at partition 0-63/64-127).
  - Attention entirely in the transposed [k, q] layout: scores^T via PE
    (contraction d=32 at row strip 32h), exp on ACT with accum_out giving the
    softmax-over-q denominators per k for free, DVE reciprocal, DVE
    tensor_scalar per-partition multiply -> normalized attn^T tiles, which
    are both DMA'd out (host transposes the view) and fed back to the PE for
    x^T[d, q] = sum_k v^T[k, d] * attn^T[k, q] per head.
  - Partial output conv over this core's 128 channels, AllReduce over core
    pairs {2b, 2b+1}, then bias + double residual + LayerNorm -> y.

Outputs per core: attnT [4, 4096, 1024] f32 and y [64, 1024] f32.
kernel() assembles the full (y, attn) tuple matching the reference.
"""

import math
import sys
import types

import numpy as np

# ---- problem constants (hardcoded; kernel.py must be self-contained) ----
N_HEAD, D_IMAGE, D_K, D_V = 8, 64, 32, 32
H, W, KS = 32, 32, 3
N_SEQ, B = 4, 4
HW = H * W                  # 1024
NK = N_SEQ * HW             # 4096
HPG = N_HEAD // 2           # heads per group = 4
NCORES = 8
SCALE = 1.0 / math.sqrt(D_K)
LN_EPS = 1e-6

_CACHE = {}


def _ensure_profile_hook():
    """Register the axon NTFF profiling hook if the antenv shim is missing."""
    if "antenv.axon_hooks" in sys.modules:
        return
    try:
        from trn_agent_boot.trn_boot import _ntff_profile_via_ctypes
        hook = _ntff_profile_via_ctypes("/opt/axon/libaxon_pjrt.so")
    except Exception:
        hook = None
    mod = types.ModuleType("antenv.axon_hooks")
    mod.get_axon_ntff_profile_hook = lambda: hook
    mod.set_axon_ntff_profile_hook = lambda h: None
    sys.modules["antenv.axon_hooks"] = mod


def _build():
    import concourse.bacc as bacc
    import concourse.tile as tile
    from concourse import mybir

    F32 = mybir.dt.float32
    F32R = mybir.dt.float32r
    BF16 = mybir.dt.bfloat16
    EXP = mybir.ActivationFunctionType.Exp
    LNF = mybir.ActivationFunctionType.Ln
    SQF = mybir.ActivationFunctionType.Square

    nc = bacc.Bacc("TRN2", target_bir_lowering=False, debug=False,
                   num_devices=NCORES)

    # ---- DRAM I/O ----
    d_qpad = nc.dram_tensor("qpad", [64, 34, 34], F32, kind="ExternalInput")
    d_s01 = nc.dram_tensor("s01", [128, 34, 34], F32, kind="ExternalInput")
    d_s23 = nc.dram_tensor("s23", [128, 34, 34], F32, kind="ExternalInput")
    d_wq = nc.dram_tensor("wq", [64, 9, 128], F32, kind="ExternalInput")
    d_wk = nc.dram_tensor("wk", [128, 9, 128], F32, kind="ExternalInput")
    d_wv = nc.dram_tensor("wv", [128, 9, 128], F32, kind="ExternalInput")
    d_wo = nc.dram_tensor("wo", [128, 9, 64], F32, kind="ExternalInput")
    d_bq = nc.dram_tensor("bq", [128, 1], F32, kind="ExternalInput")
    d_bk = nc.dram_tensor("bk", [128, 1], F32, kind="ExternalInput")
    d_bv = nc.dram_tensor("bv", [128, 1], F32, kind="ExternalInput")
    d_bo = nc.dram_tensor("bo", [64, 1], F32, kind="ExternalInput")
    d_lnw = nc.dram_tensor("lnw", [64, 1024], F32, kind="ExternalInput")
    d_lnb = nc.dram_tensor("lnb", [64, 1024], F32, kind="ExternalInput")
    d_ident = nc.dram_tensor("ident", [128, 128], F32, kind="ExternalInput")

    d_attnT = nc.dram_tensor("attnT", [HPG, NK, HW], F32,
                             kind="ExternalOutput")
    d_y = nc.dram_tensor("y", [64, 1024], F32, kind="ExternalOutput")

    d_ccin = nc.dram_tensor("ccin", [64, 1024], F32)
    d_ccout = nc.dram_tensor("ccout", [64, 1024], F32)

    with tile.TileContext(nc) as tc:
        with (
            tc.tile_pool(name="pers", bufs=1) as pers,
            tc.tile_pool(name="stage", bufs=2) as stage,
            tc.tile_pool(name="expool", bufs=2) as expool,
            tc.tile_pool(name="attnp", bufs=2) as attnp,
            tc.tile_pool(name="expbp", bufs=2) as expbp,
            tc.tile_pool(name="small", bufs=6) as small,
            tc.tile_pool(name="ps", bufs=3, space="PSUM") as ps,
            tc.tile_pool(name="xps", bufs=1, space="PSUM") as xps,
        ):
            # ---- load + round inputs ----
            def load_round(dram, shape, rdtype):
                st = stage.tile(shape, F32, tag="ldstage", name="st")
                nc.sync.dma_start(st[:], dram.ap()[:])
                rt = pers.tile(shape, rdtype, tag=dram.name + "_r",
                               name=dram.name + "_r")
                nc.vector.tensor_copy(rt[:], st[:])
                return rt

            qpad_f = pers.tile([64, 34, 34], F32, tag="qpad_f")
            nc.sync.dma_start(qpad_f[:], d_qpad.ap()[:])
            qpad = pers.tile([64, 34, 34], F32R, tag="qpad_r")
            nc.vector.tensor_copy(qpad[:], qpad_f[:])
            s01 = load_round(d_s01, [128, 34, 34], F32R)
            s23 = load_round(d_s23, [128, 34, 34], F32R)
            wq = load_round(d_wq, [64, 9, 128], F32R)
            wk = load_round(d_wk, [128, 9, 128], F32R)
            wv = load_round(d_wv, [128, 9, 128], F32R)
            ident_r = load_round(d_ident, [128, 128], F32R)

            def load_f32(dram, shape, tag):
                t = pers.tile(shape, F32, tag=tag, name=tag)
                nc.sync.dma_start(t[:], dram.ap()[:])
                return t

            wo = load_f32(d_wo, [128, 9, 64], "wo")
            bq = load_f32(d_bq, [128, 1], "bq")
            bk = load_f32(d_bk, [128, 1], "bk")
            bv = load_f32(d_bv, [128, 1], "bv")
            bo = load_f32(d_bo, [64, 1], "bo")
            lnw = load_f32(d_lnw, [64, 1024], "lnw")
            lnb = load_f32(d_lnb, [64, 1024], "lnb")

            # ---- conv: 3x3 SAME as 9 shifted matmuls ----
            def conv_mms(psum_t, w_t, img_t, ic0, pos):
                for o in range(9):
                    ky, kx = o // 3, o % 3
                    for rh in range(2):
                        nc.tensor.matmul(
                            psum_t[:, rh * 512:(rh + 1) * 512],
                            w_t[ic0:ic0 + 64, o, :],
                            img_t[ic0:ic0 + 64,
                                  ky + rh * 16: ky + rh * 16 + 16,
                                  kx: kx + 32],
                            start=(o == 0), stop=(o == 8),
                            tile_position=pos,
                        )

            # Q conv -> qconv [128, 1024] f32r
            qps = ps.tile([128, 1024], F32, tag="ps1")
            conv_mms(qps, wq, qpad, 0, (0, 0))
            qconv = pers.tile([128, 1024], F32R, tag="qconv")
            nc.vector.tensor_scalar_add(qconv[:], qps[:], bq[:])

            # K/V convs over 4 seq images (2-image row packing,
            # halves interleaved so the two row strips run concurrently)
            kconv = pers.tile([128, NK], F32R, tag="kconv")
            vconv = pers.tile([128, NK], F32R, tag="vconv")

            def conv_pair(w_t, img_t, p0, p1):
                for o in range(9):
                    ky, kx = o // 3, o % 3
                    for rh in range(2):
                        for half, pt in ((0, p0), (1, p1)):
                            nc.tensor.matmul(
                                pt[:, rh * 512:(rh + 1) * 512],
                                w_t[64 * half:64 * half + 64, o, :],
                                img_t[64 * half:64 * half + 64,
                                      ky + rh * 16: ky + rh * 16 + 16,
                                      kx: kx + 32],
                                start=(o == 0), stop=(o == 8),
                                tile_position=(64 * half, 0),
                            )

            def conv_pair_imgs(pair, img_t):
                kp0 = ps.tile([128, 1024], F32, tag="ps1", name="kp0")
                kp1 = ps.tile([128, 1024], F32, tag="ps1", name="kp1")
                conv_pair(wk, img_t, kp0, kp1)
                for half, pt in ((0, kp0), (1, kp1)):
                    img = 2 * pair + half
                    nc.vector.tensor_scalar_add(
                        kconv[:, img * 1024:(img + 1) * 1024], pt[:], bk[:])
                vp0 = ps.tile([128, 1024], F32, tag="ps1", name="vp0")
                vp1 = ps.tile([128, 1024], F32, tag="ps1", name="vp1")
                conv_pair(wv, img_t, vp0, vp1)
                for half, pt in ((0, vp0), (1, vp1)):
                    img = 2 * pair + half
                    nc.vector.tensor_scalar_add(
                        vconv[:, img * 1024:(img + 1) * 1024], pt[:], bv[:])

            def vtrans(kc):
                vtp = ps.tile([128, 128], F32R, tag="ps1", name="vtp")
                nc.tensor.transpose(
                    vtp[:], vconv[:, kc * 128:(kc + 1) * 128], ident_r[:])
                nc.vector.tensor_copy(vT[:, kc, :], vtp[:])

            vT = pers.tile([128, 32, 128], BF16, tag="vT")
            xpad = pers.tile([128, 34, 34], F32, tag="xpad")
            nc.vector.memset(xpad[:], 0.0)
            xT = xps.tile([128, 1024], F32, tag="xT", name="xT")

            def main_kc(kc):
                att_st = attnp.tile([128, HPG, 1024], F32R, tag="att",
                                    name="att_st")
                for h in range(HPG):
                    hp = 32 * h
                    sTh = ps.tile([128, 1024], F32, tag="ps1", name="sTh")
                    for qh in range(2):
                        nc.tensor.matmul(
                            sTh[:, qh * 512:(qh + 1) * 512],
                            kconv[hp:hp + 32, kc * 128:(kc + 1) * 128],
                            qconv[hp:hp + 32, qh * 512:(qh + 1) * 512],
                            start=True, stop=True,
                            tile_position=(hp, 0),
                        )
                    expT = expool.tile([128, 1024], F32, tag="expT",
                                       name="expT")
                    cs = small.tile([128, 1], F32, tag="cs", name="cs")
                    nc.scalar.activation(expT[:], sTh[:], EXP,
                                         scale=SCALE, accum_out=cs[:])
                    rs = small.tile([128, 1], F32, tag="rs", name="rs")
                    nc.vector.reciprocal(rs[:], cs[:])
                    nc.vector.tensor_scalar_mul(
                        att_st[:, h, :], expT[:], rs[:])
                    expb = expbp.tile([128, 1024], BF16, tag="expb",
                                      name="expb")
                    if h == 3:
                        nc.scalar.activation(
                            expb[:], att_st[:, h, :],
                            mybir.ActivationFunctionType.Copy)
                    else:
                        nc.vector.tensor_copy(expb[:], att_st[:, h, :])
                    for qh in range(2):
                        nc.tensor.matmul(
                            xT[hp:hp + 32, qh * 512:(qh + 1) * 512],
                            vT[:, kc, hp:hp + 32],
                            expb[:, qh * 512:(qh + 1) * 512],
                            start=(kc == 0), stop=(kc == 31),
                            tile_position=(0, hp),
                        )
                out_ap = d_attnT.ap()[
                    :, kc * 128:(kc + 1) * 128, :
                ].rearrange("h p q -> p h q")
                nc.sync.dma_start(out_ap, att_st[:].bitcast(F32))

            for pair, img_t in ((0, s01), (1, s23)):
                conv_pair_imgs(pair, img_t)
                for kc in range(16 * pair, 16 * pair + 16):
                    vtrans(kc)
                for kc in range(16 * pair, 16 * pair + 16):
                    main_kc(kc)
            nc.vector.tensor_copy(xpad[:, 1:33, 1:33], xT[:])


            # ---- output conv (partial over this core's 128 channels) ----
            ops_t = ps.tile([128, 1024], F32, tag="ps1", name="ops_t")
            for o in range(9):
                ky, kx = o // 3, o % 3
                for rh in range(2):
                    nc.tensor.matmul(
                        ops_t[0:64, rh * 512:(rh + 1) * 512],
                        wo[:, o, :],
                        xpad[:, ky + rh * 16: ky + rh * 16 + 16, kx: kx + 32],
                        start=(o == 0), stop=(o == 8),
                    )
            osb = pers.tile([64, 1024], F32, tag="osb")
            nc.vector.tensor_copy(osb[:], ops_t[0:64, :])
            nc.sync.dma_start(d_ccin.ap()[:], osb[:])
            nc.gpsimd.collective_compute(
                "AllReduce", mybir.AluOpType.add,
                replica_groups=[[0, 1], [2, 3], [4, 5], [6, 7]],
                ins=[d_ccin.ap()[:]],
                outs=[d_ccout.ap()[:]],
            )
            ofull = pers.tile([64, 1024], F32, tag="ofull")
            nc.sync.dma_start(ofull[:], d_ccout.ap()[:])

            # t = ofull + bo + 2*query
            t_t = pers.tile([64, 1024], F32, tag="t_t")
            nc.vector.tensor_scalar_add(t_t[:], ofull[:], bo[:])
            q2 = pers.tile([64, 1024], F32, tag="q2")
            nc.vector.tensor_scalar_mul(q2[:], qpad_f[:, 1:33, 1:33], 2.0)
            nc.vector.tensor_tensor(
                out=t_t[:], in0=t_t[:], in1=q2[:], op=mybir.AluOpType.add)

            # stats over all 64*1024 elements
            stats = pers.tile([64, 2], F32, tag="stats")
            nc.vector.tensor_reduce(
                stats[:, 0:1], t_t[:], axis=mybir.AxisListType.X,
                op=mybir.AluOpType.add)
            junk = pers.tile([64, 1024], F32, tag="junk")
            nc.scalar.activation(junk[:], t_t[:], SQF,
                                 accum_out=stats[:, 1:2])
            ones64 = pers.tile([64, 1], F32, tag="ones64")
            nc.vector.memset(ones64[:], 1.0)
            onesr = pers.tile([1, 64], F32, tag="onesr")
            nc.vector.memset(onesr[:], 1.0)
            stp = ps.tile([128, 1024], F32, tag="ps1", name="stp")
            nc.tensor.matmul(stp[0:1, 0:2], ones64[:], stats[:],
                             start=True, stop=True)
            mr = pers.tile([1, 2], F32, tag="mr")     # [mean, rstd]
            nc.vector.tensor_scalar_mul(mr[:, 0:1], stp[0:1, 0:1],
                                        1.0 / 65536.0)
            et2 = pers.tile([1, 1], F32, tag="et2")
            nc.vector.tensor_scalar_mul(et2[:], stp[0:1, 1:2], 1.0 / 65536.0)
            m2 = pers.tile([1, 1], F32, tag="m2")
            nc.vector.tensor_tensor(out=m2[:], in0=mr[:, 0:1], in1=mr[:, 0:1],
                                    op=mybir.AluOpType.mult)
            var = pers.tile([1, 1], F32, tag="var")
            nc.vector.tensor_tensor(out=var[:], in0=et2[:], in1=m2[:],
                                    op=mybir.AluOpType.subtract)
            lnv = pers.tile([1, 1], F32, tag="lnv")
            epst = pers.tile([1, 1], F32, tag="epst")
            nc.vector.memset(epst[:], LN_EPS)
            nc.scalar.activation(lnv[:], var[:], LNF, bias=epst[:])
            nc.scalar.activation(mr[:, 1:2], lnv[:], EXP, scale=-0.5)
            bcp = ps.tile([128, 1024], F32, tag="ps1", name="bcp")
            nc.tensor.matmul(bcp[0:64, 0:2], onesr[:, :], mr[:],
                             start=True, stop=True)
            bc = pers.tile([64, 2], F32, tag="bc")
            nc.vector.tensor_copy(bc[:], bcp[0:64, 0:2])
            # y = ((t - mean) * rstd) * lnw + lnb
            u = pers.tile([64, 1024], F32, tag="u")
            nc.vector.tensor_scalar(
                out=u[:], in0=t_t[:],
                scalar1=bc[:, 0:1], scalar2=bc[:, 1:2],
                op0=mybir.AluOpType.subtract, op1=mybir.AluOpType.mult)
            nc.vector.tensor_tensor(out=u[:], in0=u[:], in1=lnw[:],
                                    op=mybir.AluOpType.mult)
            nc.vector.tensor_tensor(out=u[:], in0=u[:], in1=lnb[:],
                                    op=mybir.AluOpType.add)
            nc.sync.dma_start(d_y.ap()[:], u[:])

    nc.compile()
    return nc


def _prep_inputs(inputs):
    """Host-side sharding/layout prep (data movement only)."""
    q = np.asarray(inputs["query_image"], np.float32)
    s = np.asarray(inputs["seq_images"], np.float32)
    wq = np.asarray(inputs["wq"], np.float32)
    bq = np.asarray(inputs["bq"], np.float32)
    wk = np.asarray(inputs["wk"], np.float32)
    bk = np.asarray(inputs["bk"], np.float32)
    wv = np.asarray(inputs["wv"], np.float32)
    bv = np.asarray(inputs["bv"], np.float32)
    wo = np.asarray(inputs["wo"], np.float32)
    bo = np.asarray(inputs["bo"], np.float32)
    lnw = np.asarray(inputs["ln_w"], np.float32).reshape(64, 1024)
    lnb = np.asarray(inputs["ln_b"], np.float32).reshape(64, 1024)
    ident = np.eye(128, dtype=np.float32)

    def pad(img):  # [C, 32, 32] -> [C, 34, 34]
        return np.pad(img, [(0, 0), (1, 1), (1, 1)])

    def wt(w_slice):  # [oc, 64ic, 3, 3] -> [ic, 9, oc]
        oc = w_slice.shape[0]
        return np.ascontiguousarray(
            w_slice.reshape(oc, 64, 9).transpose(1, 2, 0))

    in_maps = []
    for c in range(NCORES):
        b, g = c // 2, c % 2
        oc0 = g * 128
        wq_t = wt(wq[oc0:oc0 + 128])
        wk_t = wt(wk[oc0:oc0 + 128])
        wv_t = wt(wv[oc0:oc0 + 128])
        wo_t = np.ascontiguousarray(
            wo[:, oc0:oc0 + 128].reshape(64, 128, 9).transpose(1, 2, 0))
        in_maps.append({
            "qpad": pad(q[b]),
            "s01": np.concatenate([pad(s[0, b]), pad(s[1, b])], 0),
            "s23": np.concatenate([pad(s[2, b]), pad(s[3, b])], 0),
            "wq": wq_t,
            "wk": np.concatenate([wk_t, wk_t], 0),
            "wv": np.concatenate([wv_t, wv_t], 0),
            "wo": wo_t,
            "bq": bq[oc0:oc0 + 128, None],
            "bk": bk[oc0:oc0 + 128, None],
            "bv": bv[oc0:oc0 + 128, None],
            "bo": bo[:, None],
            "lnw": lnw,
            "lnb": lnb,
            "ident": ident,
        })
    return in_maps


def run(inputs, trace=False):
    """Run on the 8 cores; returns ((y, attn), BassKernelResults)."""
    _ensure_profile_hook()
    from concourse.bass_utils import run_bass_kernel_spmd

    if "nc" not in _CACHE:
        _CACHE["nc"] = _build()
    nc = _CACHE["nc"]

    in_maps = _prep_inputs(inputs)
    br = run_bass_kernel_spmd(
        nc, in_maps, core_ids=list(range(NCORES)), trace=trace)
    res = br.results

    # assemble attn: per-core attnT [4, 4096, 1024] -> attn [4, 8, 1024, 4096]
    per_b = [
        np.concatenate([res[2 * b]["attnT"], res[2 * b + 1]["attnT"]], 0)
        for b in range(B)
    ]
    attn = np.stack(per_b, 0).swapaxes(2, 3)        # view transpose
    y = np.stack([res[2 * b]["y"].reshape(64, 32, 32) for b in range(B)], 0)
    return (y, attn), br


def kernel(**inputs):
    out, _ = run(inputs, trace=False)
    return out


# revision 16
# speedup vs baseline: 1.0687x; 1.0687x over previous
"""MultiHeadImageAttentionBlock on 8 Trainium2 NeuronCores (Bass/Tile).

Sharding: core c = 2*b + g handles batch b (of 4) and head-group g (4 of 8
heads). Per core:
  - Q/K/V 3x3 convs as 9 shifted matmuls on padded 34x34 images (f32r, full
    PE rate), K/V with 2-image row-packing (images at partition 0-63/64-127).
  - Attention entirely in the transposed [k, q] layout: scores^T via PE
    (contraction d=32 at row strip 32h), exp on ACT with accum_out giving the
    softmax-over-q denominators per k for free, DVE reciprocal, DVE
    tensor_scalar per-partition multiply -> normalized attn^T tiles, which
    are both DMA'd out (host transposes the view) and fed back to the PE for
    x^T[d, q] = sum_k v^T[k, d] * attn^T[k, q] per head.
  - Partial output conv over this core's 128 channels, AllReduce over core
    pairs {2b, 2b+1}, then bias + double residual + LayerNorm -> y.

Outputs per core: attnT [4, 4096, 1024] f32 and y [64, 1024] f32.
kernel() assembles the full (y, attn) tuple matching the reference.
"""

import math
import sys
import types

import numpy as np

# ---- problem constants (hardcoded; kernel.py must be self-contained) ----
N_HEAD, D_IMAGE, D_K, D_V = 8, 64, 32, 32
H, W, KS = 32, 32, 3
N_SEQ, B = 4, 4
HW = H * W                  # 1024
NK = N_SEQ * HW             # 4096
HPG = N_HEAD // 2           # heads per group = 4
NCORES = 8
SCALE = 1.0 / math.sqrt(D_K)
LN_EPS = 1e-6

_CACHE = {}


def _ensure_profile_hook():
    """Register the axon NTFF profiling hook if the antenv shim is missing."""
    if "antenv.axon_hooks" in sys.modules:
        return
    try:
        from trn_agent_boot.trn_boot import _ntff_profile_via_ctypes
        hook = _ntff_profile_via_ctypes("/opt/axon/libaxon_pjrt.so")
    except Exception:
        hook = None
    mod = types.ModuleType("antenv.axon_hooks")
    mod.get_axon_ntff_profile_hook = lambda: hook
    mod.set_axon_ntff_profile_hook = lambda h: None
    sys.modules["antenv.axon_hooks"] = mod


def _build():
    import concourse.bacc as bacc
    import concourse.tile as tile
    from concourse import mybir

    F32 = mybir.dt.float32
    F32R = mybir.dt.float32r
    BF16 = mybir.dt.bfloat16
    EXP = mybir.ActivationFunctionType.Exp
    LNF = mybir.ActivationFunctionType.Ln
    SQF = mybir.ActivationFunctionType.Square

    nc = bacc.Bacc("TRN2", target_bir_lowering=False, debug=False,
                   num_devices=NCORES)

    # ---- DRAM I/O ----
    d_qpad = nc.dram_tensor("qpad", [64, 34, 34], F32, kind="ExternalInput")
    d_s01 = nc.dram_tensor("s01", [128, 34, 34], F32, kind="ExternalInput")
    d_s23 = nc.dram_tensor("s23", [128, 34, 34], F32, kind="ExternalInput")
    d_wq = nc.dram_tensor("wq", [64, 9, 128], F32, kind="ExternalInput")
    d_wk = nc.dram_tensor("wk", [128, 9, 128], F32, kind="ExternalInput")
    d_wv = nc.dram_tensor("wv", [128, 9, 128], F32, kind="ExternalInput")
    d_wo = nc.dram_tensor("wo", [128, 9, 64], F32, kind="ExternalInput")
    d_bq = nc.dram_tensor("bq", [128, 1], F32, kind="ExternalInput")
    d_bk = nc.dram_tensor("bk", [128, 1], F32, kind="ExternalInput")
    d_bv = nc.dram_tensor("bv", [128, 1], F32, kind="ExternalInput")
    d_bo = nc.dram_tensor("bo", [64, 1], F32, kind="ExternalInput")
    d_lnw = nc.dram_tensor("lnw", [64, 1024], F32, kind="ExternalInput")
    d_lnb = nc.dram_tensor("lnb", [64, 1024], F32, kind="ExternalInput")
    d_ident = nc.dram_tensor("ident", [128, 128], F32, kind="ExternalInput")

    d_attnT = nc.dram_tensor("attnT", [HPG, NK, HW], F32,
                             kind="ExternalOutput")
    d_y = nc.dram_tensor("y", [64, 1024], F32, kind="ExternalOutput")

    d_ccin = nc.dram_tensor("ccin", [64, 1024], F32)
    d_ccout = nc.dram_tensor("ccout", [64, 1024], F32)

    with tile.TileContext(nc) as tc:
        with (
            tc.tile_pool(name="pers", bufs=1) as pers,
            tc.tile_pool(name="stage", bufs=2) as stage,
            tc.tile_pool(name="expool", bufs=2) as expool,
            tc.tile_pool(name="attnp", bufs=2) as attnp,
            tc.tile_pool(name="expbp", bufs=2) as expbp,
            tc.tile_pool(name="small", bufs=6) as small,
            tc.tile_pool(name="ps", bufs=3, space="PSUM") as ps,
            tc.tile_pool(name="xps", bufs=1, space="PSUM") as xps,
        ):
            # ---- load + round inputs ----
            def load_round(dram, shape, rdtype):
                st = stage.tile(shape, F32, tag="ldstage", name="st")
                nc.sync.dma_start(st[:], dram.ap()[:])
                rt = pers.tile(shape, rdtype, tag=dram.name + "_r",
                               name=dram.name + "_r")
                nc.vector.tensor_copy(rt[:], st[:])
                return rt

            qpad_f = pers.tile([64, 34, 34], F32, tag="qpad_f")
            nc.sync.dma_start(qpad_f[:], d_qpad.ap()[:])
            qpad = pers.tile([64, 34, 34], F32R, tag="qpad_r")
            nc.vector.tensor_copy(qpad[:], qpad_f[:])
            s01 = load_round(d_s01, [128, 34, 34], F32R)
            s23 = load_round(d_s23, [128, 34, 34], F32R)
            wq = load_round(d_wq, [64, 9, 128], F32R)
            wk = load_round(d_wk, [128, 9, 128], F32R)
            wv = load_round(d_wv, [128, 9, 128], F32R)
            ident_r = load_round(d_ident, [128, 128], F32R)

            def load_f32(dram, shape, tag):
                t = pers.tile(shape, F32, tag=tag, name=tag)
                nc.sync.dma_start(t[:], dram.ap()[:])
                return t

            wo = load_f32(d_wo, [128, 9, 64], "wo")
            bq = load_f32(d_bq, [128, 1], "bq")
            bk = load_f32(d_bk, [128, 1], "bk")
            bv = load_f32(d_bv, [128, 1], "bv")
            bo = load_f32(d_bo, [64, 1], "bo")
            lnw = load_f32(d_lnw, [64, 1024], "lnw")
            lnb = load_f32(d_lnb, [64, 1024], "lnb")

            # ---- conv: 3x3 SAME as 9 shifted matmuls ----
            def conv_mms(psum_t, w_t, img_t, ic0, pos):
                for o in range(9):
                    ky, kx = o // 3, o % 3
                    for rh in range(2):
                        nc.tensor.matmul(
                            psum_t[:, rh * 512:(rh + 1) * 512],
                            w_t[ic0:ic0 + 64, o, :],
                            img_t[ic0:ic0 + 64,
                                  ky + rh * 16: ky + rh * 16 + 16,
                                  kx: kx + 32],
                            start=(o == 0), stop=(o == 8),
                            tile_position=pos,
                        )

            # Q conv -> qconv [128, 1024] f32r
            qps = ps.tile([128, 1024], F32, tag="ps1")
            conv_mms(qps, wq, qpad, 0, (0, 0))
            qconv = pers.tile([128, 1024], F32R, tag="qconv")
            nc.vector.tensor_scalar_add(qconv[:], qps[:], bq[:])

            # K/V convs over 4 seq images (2-image row packing,
            # halves interleaved so the two row strips run concurrently)
            kconv = pers.tile([128, NK], F32R, tag="kconv")
            vconv = pers.tile([128, NK], F32R, tag="vconv")

            def conv_pair(w_t, img_t, p0, p1):
                for o in range(9):
                    ky, kx = o // 3, o % 3
                    for rh in range(2):
                        for half, pt in ((0, p0), (1, p1)):
                            nc.tensor.matmul(
                                pt[:, rh * 512:(rh + 1) * 512],
                                w_t[64 * half:64 * half + 64, o, :],
                                img_t[64 * half:64 * half + 64,
                                      ky + rh * 16: ky + rh * 16 + 16,
                                      kx: kx + 32],
                                start=(o == 0), stop=(o == 8),
                                tile_position=(64 * half, 0),
                            )

            def conv_pair_imgs(pair, img_t):
                kp0 = ps.tile([128, 1024], F32, tag="ps1", name="kp0")
                kp1 = ps.tile([128, 1024], F32, tag="ps1", name="kp1")
                conv_pair(wk, img_t, kp0, kp1)
                for half, pt in ((0, kp0), (1, kp1)):
                    img = 2 * pair + half
                    nc.vector.tensor_scalar_add(
                        kconv[:, img * 1024:(img + 1) * 1024], pt[:], bk[:])
                vp0 = ps.tile([128, 1024], F32, tag="ps1", name="vp0")
                vp1 = ps.tile([128, 1024], F32, tag="ps1", name="vp1")
                conv_pair(wv, img_t, vp0, vp1)
                for half, pt in ((0, vp0), (1, vp1)):
                    img = 2 * pair + half
                    nc.vector.tensor_scalar_add(
                        vconv[:, img * 1024:(img + 1) * 1024], pt[:], bv[:])

            def vtrans(kc):
                vtp = ps.tile([128, 128], F32R, tag="ps1", name="vtp")
                nc.tensor.transpose(
                    vtp[:], vconv[:, kc * 128:(kc + 1) * 128], ident_r[:])
                nc.vector.tensor_copy(vT[:, kc, :], vtp[:])

            vT = pers.tile([128, 32, 128], BF16, tag="vT")
            xpad = pers.tile([128, 34, 34], F32, tag="xpad")
            nc.vector.memset(xpad[:], 0.0)
            xT = xps.tile([128, 1024], F32, tag="xT", name="xT")

            def main_kc(kc):
                att_st = attnp.tile([128, HPG, 1024], F32R, tag="att",
                                    name="att_st")
                for h in range(HPG):
                    hp = 32 * h
                    sTh = ps.tile([128, 1024], F32, tag="ps1", name="sTh")
                    for qh in range(2):
                        nc.tensor.matmul(
                            sTh[:, qh * 512:(qh + 1) * 512],
                            kconv[hp:hp + 32, kc * 128:(kc + 1) * 128],
                            qconv[hp:hp + 32, qh * 512:(qh + 1) * 512],
                            start=True, stop=True,
                            tile_position=(hp, 0),
                        )
                    expT = expool.tile([128, 1024], F32, tag="expT",
                                       name="expT")
                    cs = small.tile([128, 1], F32, tag="cs", name="cs")
                    nc.scalar.activation(expT[:], sTh[:], EXP,
                                         scale=SCALE, accum_out=cs[:])
                    rs = small.tile([128, 1], F32, tag="rs", name="rs")
                    nc.vector.reciprocal(rs[:], cs[:])
                    nc.vector.tensor_scalar_mul(
                        att_st[:, h, :], expT[:], rs[:])
                    expb = expbp.tile([128, 1024], BF16, tag="expb",
                                      name="expb")
                    nc.vector.tensor_copy(expb[:], att_st[:, h, :])
                    for qh in range(2):
                        nc.tensor.matmul(
                            xT[hp:hp + 32, qh * 512:(qh + 1) * 512],
                            vT[:, kc, hp:hp + 32],
                            expb[:, qh * 512:(qh + 1) * 512],
                            start=(kc == 0), stop=(kc == 31),
                            tile_position=(0, hp),
                        )
                out_ap = d_attnT.ap()[
                    :, kc * 128:(kc + 1) * 128, :
                ].rearrange("h p q -> p h q")
                nc.sync.dma_start(out_ap, att_st[:].bitcast(F32))

            for pair, img_t in ((0, s01), (1, s23)):
                conv_pair_imgs(pair, img_t)
            for kc in range(32):
                vtrans(kc)
            for kc in range(32):
                main_kc(kc)
            nc.vector.tensor_copy(xpad[:, 1:33, 1:33], xT[:])


            # ---- output conv (partial over this core's 128 channels) ----
            ops_t = ps.tile([128, 1024], F32, tag="ps1", name="ops_t")
            for o in range(9):
                ky, kx = o // 3, o % 3
                for rh in range(2):
                    nc.tensor.matmul(
                        ops_t[0:64, rh * 512:(rh + 1) * 512],
                        wo[:, o, :],
                        xpad[:, ky + rh * 16: ky + rh * 16 + 16, kx: kx + 32],
                        start=(o == 0), stop=(o == 8),
                    )
            osb = pers.tile([64, 1024], F32, tag="osb")
            nc.vector.tensor_copy(osb[:], ops_t[0:64, :])
            nc.sync.dma_start(d_ccin.ap()[:], osb[:])
            nc.gpsimd.collective_compute(
                "AllReduce", mybir.AluOpType.add,
                replica_groups=[[0, 1], [2, 3], [4, 5], [6, 7]],
                ins=[d_ccin.ap()[:]],
                outs=[d_ccout.ap()[:]],
            )
            ofull = pers.tile([64, 1024], F32, tag="ofull")
            nc.sync.dma_start(ofull[:], d_ccout.ap()[:])

            # t = ofull + bo + 2*query
            t_t = pers.tile([64, 1024], F32, tag="t_t")
            nc.vector.tensor_scalar_add(t_t[:], ofull[:], bo[:])
            q2 = pers.tile([64, 1024], F32, tag="q2")
            nc.vector.tensor_scalar_mul(q2[:], qpad_f[:, 1:33, 1:33], 2.0)
            nc.vector.tensor_tensor(
                out=t_t[:], in0=t_t[:], in1=q2[:], op=mybir.AluOpType.add)

            # stats over all 64*1024 elements
            stats = pers.tile([64, 2], F32, tag="stats")
            nc.vector.tensor_reduce(
                stats[:, 0:1], t_t[:], axis=mybir.AxisListType.X,
                op=mybir.AluOpType.add)
            junk = pers.tile([64, 1024], F32, tag="junk")
            nc.scalar.activation(junk[:], t_t[:], SQF,
                                 accum_out=stats[:, 1:2])
            ones64 = pers.tile([64, 1], F32, tag="ones64")
            nc.vector.memset(ones64[:], 1.0)
            onesr = pers.tile([1, 64], F32, tag="onesr")
            nc.vector.memset(onesr[:], 1.0)
            stp = ps.tile([128, 1024], F32, tag="ps1", name="stp")
            nc.tensor.matmul(stp[0:1, 0:2], ones64[:], stats[:],
                             start=True, stop=True)
            mr = pers.tile([1, 2], F32, tag="mr")     # [mean, rstd]
            nc.vector.tensor_scalar_mul(mr[:, 0:1], stp[0:1, 0:1],
                                        1.0 / 65536.0)
            et2 = pers.tile([1, 1], F32, tag="et2")
            nc.vector.tensor_scalar_mul(et2[:], stp[0:1, 1:2], 1.0 / 65536.0)
            m2 = pers.tile([1, 1], F32, tag="m2")
            nc.vector.tensor_tensor(out=m2[:], in0=mr[:, 0:1], in1=mr[:, 0:1],
                                    op=mybir.AluOpType.mult)
            var = pers.tile([1, 1], F32, tag="var")
            nc.vector.tensor_tensor(out=var[:], in0=et2[:], in1=m2[:],
                                    op=mybir.AluOpType.subtract)
            lnv = pers.tile([1, 1], F32, tag="lnv")
            epst = pers.tile([1, 1], F32, tag="epst")
            nc.vector.memset(epst[:], LN_EPS)
            nc.scalar.activation(lnv[:], var[:], LNF, bias=epst[:])
            nc.scalar.activation(mr[:, 1:2], lnv[:], EXP, scale=-0.5)
            bcp = ps.tile([128, 1024], F32, tag="ps1", name="bcp")
            nc.tensor.matmul(bcp[0:64, 0:2], onesr[:, :], mr[:],
                             start=True, stop=True)
            bc = pers.tile([64, 2], F32, tag="bc")
            nc.vector.tensor_copy(bc[:], bcp[0:64, 0:2])
            # y = ((t - mean) * rstd) * lnw + lnb
            u = pers.tile([64, 1024], F32, tag="u")
            nc.vector.tensor_scalar(
                out=u[:], in0=t_t[:],
                scalar1=bc[:, 0:1], scalar2=bc[:, 1:2],
                op0=mybir.AluOpType.subtract, op1=mybir.AluOpType.mult)
            nc.vector.tensor_tensor(out=u[:], in0=u[:], in1=lnw[:],
                                    op=mybir.AluOpType.mult)
            nc.vector.tensor_tensor(out=u[:], in0=u[:], in1=lnb[:],
                                    op=mybir.AluOpType.add)
            nc.sync.dma_start(d_y.ap()[:], u[:])

    nc.compile()
    return nc


def _prep_inputs(inputs):
    """Host-side sharding/layout prep (data movement only)."""
    q = np.asarray(inputs["query_image"], np.float32)
    s = np.asarray(inputs["seq_images"], np.float32)
    wq = np.asarray(inputs["wq"], np.float32)
    bq = np.asarray(inputs["bq"], np.float32)
    wk = np.asarray(inputs["wk"], np.float32)
    bk = np.asarray(inputs["bk"], np.float32)
    wv = np.asarray(inputs["wv"], np.float32)
    bv = np.asarray(inputs["bv"], np.float32)
    wo = np.asarray(inputs["wo"], np.float32)
    bo = np.asarray(inputs["bo"], np.float32)
    lnw = np.asarray(inputs["ln_w"], np.float32).reshape(64, 1024)
    lnb = np.asarray(inputs["ln_b"], np.float32).reshape(64, 1024)
    ident = np.eye(128, dtype=np.float32)

    def pad(img):  # [C, 32, 32] -> [C, 34, 34]
        return np.pad(img, [(0, 0), (1, 1), (1, 1)])

    def wt(w_slice):  # [oc, 64ic, 3, 3] -> [ic, 9, oc]
        oc = w_slice.shape[0]
        return np.ascontiguousarray(
            w_slice.reshape(oc, 64, 9).transpose(1, 2, 0))

    in_maps = []
    for c in range(NCORES):
        b, g = c // 2, c % 2
        oc0 = g * 128
        wq_t = wt(wq[oc0:oc0 + 128])
        wk_t = wt(wk[oc0:oc0 + 128])
        wv_t = wt(wv[oc0:oc0 + 128])
        wo_t = np.ascontiguousarray(
            wo[:, oc0:oc0 + 128].reshape(64, 128, 9).transpose(1, 2, 0))
        in_maps.append({
            "qpad": pad(q[b]),
            "s01": np.concatenate([pad(s[0, b]), pad(s[1, b])], 0),
            "s23": np.concatenate([pad(s[2, b]), pad(s[3, b])], 0),
            "wq": wq_t,
            "wk": np.concatenate([wk_t, wk_t], 0),
            "wv": np.concatenate([wv_t, wv_t], 0),
            "wo": wo_t,
            "bq": bq[oc0:oc0 + 128, None],
            "bk": bk[oc0:oc0 + 128, None],
            "bv": bv[oc0:oc0 + 128, None],
            "bo": bo[:, None],
            "lnw": lnw,
            "lnb": lnb,
            "ident": ident,
        })
    return in_maps


def run(inputs, trace=False):
    """Run on the 8 cores; returns ((y, attn), BassKernelResults)."""
    _ensure_profile_hook()
    from concourse.bass_utils import run_bass_kernel_spmd

    if "nc" not in _CACHE:
        _CACHE["nc"] = _build()
    nc = _CACHE["nc"]

    in_maps = _prep_inputs(inputs)
    br = run_bass_kernel_spmd(
        nc, in_maps, core_ids=list(range(NCORES)), trace=trace)
    res = br.results

    # assemble attn: per-core attnT [4, 4096, 1024] -> attn [4, 8, 1024, 4096]
    per_b = [
        np.concatenate([res[2 * b]["attnT"], res[2 * b + 1]["attnT"]], 0)
        for b in range(B)
    ]
    attn = np.stack(per_b, 0).swapaxes(2, 3)        # view transpose
    y = np.stack([res[2 * b]["y"].reshape(64, 32, 32) for b in range(B)], 0)
    return (y, attn), br


def kernel(**inputs):
    out, _ = run(inputs, trace=False)
    return out


# revision 17
# speedup vs baseline: 1.1360x; 1.0629x over previous
"""MultiHeadImageAttentionBlock on 8 Trainium2 NeuronCores (Bass/Tile).

Sharding: core c = 2*b + g handles batch b (of 4) and head-group g (4 of 8
heads). Per core:
  - Q/K/V 3x3 convs as 9 shifted matmuls on padded 34x34 images (f32r, full
    PE rate), K/V with 2-image row-packing (images at partition 0-63/64-127).
  - Attention entirely in the transposed [k, q] layout: scores^T via PE
    (contraction d=32 at row strip 32h), exp on ACT with accum_out giving the
    softmax-over-q denominators per k for free, DVE reciprocal, DVE
    tensor_scalar per-partition multiply -> normalized attn^T tiles, which
    are both DMA'd out (host transposes the view) and fed back to the PE for
    x^T[d, q] = sum_k v^T[k, d] * attn^T[k, q] per head.
  - Partial output conv over this core's 128 channels, AllReduce over core
    pairs {2b, 2b+1}, then bias + double residual + LayerNorm -> y.

Outputs per core: attnT [4, 4096, 1024] f32 and y [64, 1024] f32.
kernel() assembles the full (y, attn) tuple matching the reference.
"""

import math
import sys
import types

import numpy as np

# ---- problem constants (hardcoded; kernel.py must be self-contained) ----
N_HEAD, D_IMAGE, D_K, D_V = 8, 64, 32, 32
H, W, KS = 32, 32, 3
N_SEQ, B = 4, 4
HW = H * W                  # 1024
NK = N_SEQ * HW             # 4096
HPG = N_HEAD // 2           # heads per group = 4
NCORES = 8
SCALE = 1.0 / math.sqrt(D_K)
LN_EPS = 1e-6

_CACHE = {}


def _ensure_profile_hook():
    """Register the axon NTFF profiling hook if the antenv shim is missing."""
    if "antenv.axon_hooks" in sys.modules:
        return
    try:
        from trn_agent_boot.trn_boot import _ntff_profile_via_ctypes
        hook = _ntff_profile_via_ctypes("/opt/axon/libaxon_pjrt.so")
    except Exception:
        hook = None
    mod = types.ModuleType("antenv.axon_hooks")
    mod.get_axon_ntff_profile_hook = lambda: hook
    mod.set_axon_ntff_profile_hook = lambda h: None
    sys.modules["antenv.axon_hooks"] = mod


def _build():
    import concourse.bacc as bacc
    import concourse.tile as tile
    from concourse import mybir

    F32 = mybir.dt.float32
    F32R = mybir.dt.float32r
    BF16 = mybir.dt.bfloat16
    FP16 = mybir.dt.float16
    EXP = mybir.ActivationFunctionType.Exp
    LNF = mybir.ActivationFunctionType.Ln
    SQF = mybir.ActivationFunctionType.Square

    nc = bacc.Bacc("TRN2", target_bir_lowering=False, debug=False,
                   num_devices=NCORES)

    # ---- DRAM I/O ----
    d_qpad = nc.dram_tensor("qpad", [64, 34, 34], F32, kind="ExternalInput")
    d_s01 = nc.dram_tensor("s01", [128, 34, 34], F32, kind="ExternalInput")
    d_s23 = nc.dram_tensor("s23", [128, 34, 34], F32, kind="ExternalInput")
    d_wq = nc.dram_tensor("wq", [64, 9, 128], F32, kind="ExternalInput")
    d_wk = nc.dram_tensor("wk", [128, 9, 128], F32, kind="ExternalInput")
    d_wv = nc.dram_tensor("wv", [128, 9, 128], F32, kind="ExternalInput")
    d_wo = nc.dram_tensor("wo", [128, 9, 64], F32, kind="ExternalInput")
    d_bq = nc.dram_tensor("bq", [128, 1], F32, kind="ExternalInput")
    d_bk = nc.dram_tensor("bk", [128, 1], F32, kind="ExternalInput")
    d_bv = nc.dram_tensor("bv", [128, 1], F32, kind="ExternalInput")
    d_bo = nc.dram_tensor("bo", [64, 1], F32, kind="ExternalInput")
    d_lnw = nc.dram_tensor("lnw", [64, 1024], F32, kind="ExternalInput")
    d_lnb = nc.dram_tensor("lnb", [64, 1024], F32, kind="ExternalInput")
    d_ident = nc.dram_tensor("ident", [128, 128], F32, kind="ExternalInput")

    d_attnT = nc.dram_tensor("attnT", [HPG, NK, HW], F32,
                             kind="ExternalOutput")
    d_y = nc.dram_tensor("y", [64, 1024], F32, kind="ExternalOutput")

    d_ccin = nc.dram_tensor("ccin", [64, 1024], F32)
    d_ccout = nc.dram_tensor("ccout", [64, 1024], F32)

    with tile.TileContext(nc) as tc:
        with (
            tc.tile_pool(name="pers", bufs=1) as pers,
            tc.tile_pool(name="stage", bufs=2) as stage,
            tc.tile_pool(name="expool", bufs=2) as expool,
            tc.tile_pool(name="attnp", bufs=2) as attnp,
            tc.tile_pool(name="expbp", bufs=2) as expbp,
            tc.tile_pool(name="small", bufs=6) as small,
            tc.tile_pool(name="ps", bufs=3, space="PSUM") as ps,
            tc.tile_pool(name="xps", bufs=1, space="PSUM") as xps,
        ):
            # ---- load + round inputs ----
            def load_round(dram, shape, rdtype):
                st = stage.tile(shape, F32, tag="ldstage", name="st")
                nc.sync.dma_start(st[:], dram.ap()[:])
                rt = pers.tile(shape, rdtype, tag=dram.name + "_r",
                               name=dram.name + "_r")
                nc.vector.tensor_copy(rt[:], st[:])
                return rt

            qpad_f = pers.tile([64, 34, 34], F32, tag="qpad_f")
            nc.sync.dma_start(qpad_f[:], d_qpad.ap()[:])
            qpad = pers.tile([64, 34, 34], F32R, tag="qpad_r")
            nc.vector.tensor_copy(qpad[:], qpad_f[:])
            s01 = load_round(d_s01, [128, 34, 34], F32R)
            s23 = load_round(d_s23, [128, 34, 34], F32R)
            wq = load_round(d_wq, [64, 9, 128], F32R)
            wk = load_round(d_wk, [128, 9, 128], F32R)
            wv = load_round(d_wv, [128, 9, 128], F32R)
            ident_r = load_round(d_ident, [128, 128], F32R)

            def load_f32(dram, shape, tag):
                t = pers.tile(shape, F32, tag=tag, name=tag)
                nc.sync.dma_start(t[:], dram.ap()[:])
                return t

            wo = load_f32(d_wo, [128, 9, 64], "wo")
            bq = load_f32(d_bq, [128, 1], "bq")
            bk = load_f32(d_bk, [128, 1], "bk")
            bv = load_f32(d_bv, [128, 1], "bv")
            bo = load_f32(d_bo, [64, 1], "bo")
            lnw = load_f32(d_lnw, [64, 1024], "lnw")
            lnb = load_f32(d_lnb, [64, 1024], "lnb")

            # ---- conv: 3x3 SAME as 9 shifted matmuls ----
            def conv_mms(psum_t, w_t, img_t, ic0, pos):
                for o in range(9):
                    ky, kx = o // 3, o % 3
                    for rh in range(2):
                        nc.tensor.matmul(
                            psum_t[:, rh * 512:(rh + 1) * 512],
                            w_t[ic0:ic0 + 64, o, :],
                            img_t[ic0:ic0 + 64,
                                  ky + rh * 16: ky + rh * 16 + 16,
                                  kx: kx + 32],
                            start=(o == 0), stop=(o == 8),
                            tile_position=pos,
                        )

            # Q conv -> qconv [128, 1024] f32r
            qps = ps.tile([128, 1024], F32, tag="ps1")
            conv_mms(qps, wq, qpad, 0, (0, 0))
            qconv = pers.tile([128, 1024], FP16, tag="qconv")
            nc.vector.tensor_scalar_add(qconv[:], qps[:], bq[:])

            # K/V convs over 4 seq images (2-image row packing,
            # halves interleaved so the two row strips run concurrently)
            kconv = pers.tile([128, NK], FP16, tag="kconv")
            vconv = pers.tile([128, NK], F32R, tag="vconv")

            def conv_pair(w_t, img_t, p0, p1):
                for o in range(9):
                    ky, kx = o // 3, o % 3
                    for rh in range(2):
                        for half, pt in ((0, p0), (1, p1)):
                            nc.tensor.matmul(
                                pt[:, rh * 512:(rh + 1) * 512],
                                w_t[64 * half:64 * half + 64, o, :],
                                img_t[64 * half:64 * half + 64,
                                      ky + rh * 16: ky + rh * 16 + 16,
                                      kx: kx + 32],
                                start=(o == 0), stop=(o == 8),
                                tile_position=(64 * half, 0),
                            )

            def conv_pair_imgs(pair, img_t):
                kp0 = ps.tile([128, 1024], F32, tag="ps1", name="kp0")
                kp1 = ps.tile([128, 1024], F32, tag="ps1", name="kp1")
                conv_pair(wk, img_t, kp0, kp1)
                for half, pt in ((0, kp0), (1, kp1)):
                    img = 2 * pair + half
                    nc.vector.tensor_scalar_add(
                        kconv[:, img * 1024:(img + 1) * 1024], pt[:], bk[:])
                vp0 = ps.tile([128, 1024], F32, tag="ps1", name="vp0")
                vp1 = ps.tile([128, 1024], F32, tag="ps1", name="vp1")
                conv_pair(wv, img_t, vp0, vp1)
                for half, pt in ((0, vp0), (1, vp1)):
                    img = 2 * pair + half
                    nc.vector.tensor_scalar_add(
                        vconv[:, img * 1024:(img + 1) * 1024], pt[:], bv[:])

            def vtrans(kc):
                vtp = ps.tile([128, 128], F32R, tag="ps1", name="vtp")
                nc.tensor.transpose(
                    vtp[:], vconv[:, kc * 128:(kc + 1) * 128], ident_r[:])
                nc.vector.tensor_copy(vT[:, kc, :], vtp[:])

            vT = pers.tile([128, 32, 128], BF16, tag="vT")
            xpad = pers.tile([128, 34, 34], F32, tag="xpad")
            nc.vector.memset(xpad[:], 0.0)
            xT = xps.tile([128, 1024], F32, tag="xT", name="xT")

            def main_kc(kc):
                att_st = attnp.tile([128, HPG, 1024], F32R, tag="att",
                                    name="att_st")
                for h in range(HPG):
                    hp = 32 * h
                    sTh = ps.tile([128, 1024], F32, tag="ps1", name="sTh")
                    for qh in range(2):
                        nc.tensor.matmul(
                            sTh[:, qh * 512:(qh + 1) * 512],
                            kconv[hp:hp + 32, kc * 128:(kc + 1) * 128],
                            qconv[hp:hp + 32, qh * 512:(qh + 1) * 512],
                            start=True, stop=True,
                            tile_position=(hp, 0),
                        )
                    expT = expool.tile([128, 1024], F32, tag="expT",
                                       name="expT")
                    cs = small.tile([128, 1], F32, tag="cs", name="cs")
                    nc.scalar.activation(expT[:], sTh[:], EXP,
                                         scale=SCALE, accum_out=cs[:])
                    rs = small.tile([128, 1], F32, tag="rs", name="rs")
                    nc.vector.reciprocal(rs[:], cs[:])
                    nc.vector.tensor_scalar_mul(
                        att_st[:, h, :], expT[:], rs[:])
                    expb = expbp.tile([128, 1024], BF16, tag="expb",
                                      name="expb")
                    nc.vector.tensor_copy(expb[:], att_st[:, h, :])
                    for qh in range(2):
                        nc.tensor.matmul(
                            xT[hp:hp + 32, qh * 512:(qh + 1) * 512],
                            vT[:, kc, hp:hp + 32],
                            expb[:, qh * 512:(qh + 1) * 512],
                            start=(kc == 0), stop=(kc == 31),
                            tile_position=(0, hp),
                        )
                out_ap = d_attnT.ap()[
                    :, kc * 128:(kc + 1) * 128, :
                ].rearrange("h p q -> p h q")
                nc.sync.dma_start(out_ap, att_st[:].bitcast(F32))

            for pair, img_t in ((0, s01), (1, s23)):
                conv_pair_imgs(pair, img_t)
            for kc in range(32):
                vtrans(kc)
            for kc in range(32):
                main_kc(kc)
            nc.vector.tensor_copy(xpad[:, 1:33, 1:33], xT[:])


            # ---- output conv (partial over this core's 128 channels) ----
            ops_t = ps.tile([128, 1024], F32, tag="ps1", name="ops_t")
            for o in range(9):
                ky, kx = o // 3, o % 3
                for rh in range(2):
                    nc.tensor.matmul(
                        ops_t[0:64, rh * 512:(rh + 1) * 512],
                        wo[:, o, :],
                        xpad[:, ky + rh * 16: ky + rh * 16 + 16, kx: kx + 32],
                        start=(o == 0), stop=(o == 8),
                    )
            osb = pers.tile([64, 1024], F32, tag="osb")
            nc.vector.tensor_copy(osb[:], ops_t[0:64, :])
            nc.sync.dma_start(d_ccin.ap()[:], osb[:])
            nc.gpsimd.collective_compute(
                "AllReduce", mybir.AluOpType.add,
                replica_groups=[[0, 1], [2, 3], [4, 5], [6, 7]],
                ins=[d_ccin.ap()[:]],
                outs=[d_ccout.ap()[:]],
            )
            ofull = pers.tile([64, 1024], F32, tag="ofull")
            nc.sync.dma_start(ofull[:], d_ccout.ap()[:])

            # t = ofull + bo + 2*query
            t_t = pers.tile([64, 1024], F32, tag="t_t")
            nc.vector.tensor_scalar_add(t_t[:], ofull[:], bo[:])
            q2 = pers.tile([64, 1024], F32, tag="q2")
            nc.vector.tensor_scalar_mul(q2[:], qpad_f[:, 1:33, 1:33], 2.0)
            nc.vector.tensor_tensor(
                out=t_t[:], in0=t_t[:], in1=q2[:], op=mybir.AluOpType.add)

            # stats over all 64*1024 elements
            stats = pers.tile([64, 2], F32, tag="stats")
            nc.vector.tensor_reduce(
                stats[:, 0:1], t_t[:], axis=mybir.AxisListType.X,
                op=mybir.AluOpType.add)
            junk = pers.tile([64, 1024], F32, tag="junk")
            nc.scalar.activation(junk[:], t_t[:], SQF,
                                 accum_out=stats[:, 1:2])
            ones64 = pers.tile([64, 1], F32, tag="ones64")
            nc.vector.memset(ones64[:], 1.0)
            onesr = pers.tile([1, 64], F32, tag="onesr")
            nc.vector.memset(onesr[:], 1.0)
            stp = ps.tile([128, 1024], F32, tag="ps1", name="stp")
            nc.tensor.matmul(stp[0:1, 0:2], ones64[:], stats[:],
                             start=True, stop=True)
            mr = pers.tile([1, 2], F32, tag="mr")     # [mean, rstd]
            nc.vector.tensor_scalar_mul(mr[:, 0:1], stp[0:1, 0:1],
                                        1.0 / 65536.0)
            et2 = pers.tile([1, 1], F32, tag="et2")
            nc.vector.tensor_scalar_mul(et2[:], stp[0:1, 1:2], 1.0 / 65536.0)
            m2 = pers.tile([1, 1], F32, tag="m2")
            nc.vector.tensor_tensor(out=m2[:], in0=mr[:, 0:1], in1=mr[:, 0:1],
                                    op=mybir.AluOpType.mult)
            var = pers.tile([1, 1], F32, tag="var")
            nc.vector.tensor_tensor(out=var[:], in0=et2[:], in1=m2[:],
                                    op=mybir.AluOpType.subtract)
            lnv = pers.tile([1, 1], F32, tag="lnv")
            epst = pers.tile([1, 1], F32, tag="epst")
            nc.vector.memset(epst[:], LN_EPS)
            nc.scalar.activation(lnv[:], var[:], LNF, bias=epst[:])
            nc.scalar.activation(mr[:, 1:2], lnv[:], EXP, scale=-0.5)
            bcp = ps.tile([128, 1024], F32, tag="ps1", name="bcp")
            nc.tensor.matmul(bcp[0:64, 0:2], onesr[:, :], mr[:],
                             start=True, stop=True)
            bc = pers.tile([64, 2], F32, tag="bc")
            nc.vector.tensor_copy(bc[:], bcp[0:64, 0:2])
            # y = ((t - mean) * rstd) * lnw + lnb
            u = pers.tile([64, 1024], F32, tag="u")
            nc.vector.tensor_scalar(
                out=u[:], in0=t_t[:],
                scalar1=bc[:, 0:1], scalar2=bc[:, 1:2],
                op0=mybir.AluOpType.subtract, op1=mybir.AluOpType.mult)
            nc.vector.tensor_tensor(out=u[:], in0=u[:], in1=lnw[:],
                                    op=mybir.AluOpType.mult)
            nc.vector.tensor_tensor(out=u[:], in0=u[:], in1=lnb[:],
                                    op=mybir.AluOpType.add)
            nc.sync.dma_start(d_y.ap()[:], u[:])

    nc.compile()
    return nc


def _prep_inputs(inputs):
    """Host-side sharding/layout prep (data movement only)."""
    q = np.asarray(inputs["query_image"], np.float32)
    s = np.asarray(inputs["seq_images"], np.float32)
    wq = np.asarray(inputs["wq"], np.float32)
    bq = np.asarray(inputs["bq"], np.float32)
    wk = np.asarray(inputs["wk"], np.float32)
    bk = np.asarray(inputs["bk"], np.float32)
    wv = np.asarray(inputs["wv"], np.float32)
    bv = np.asarray(inputs["bv"], np.float32)
    wo = np.asarray(inputs["wo"], np.float32)
    bo = np.asarray(inputs["bo"], np.float32)
    lnw = np.asarray(inputs["ln_w"], np.float32).reshape(64, 1024)
    lnb = np.asarray(inputs["ln_b"], np.float32).reshape(64, 1024)
    ident = np.eye(128, dtype=np.float32)

    def pad(img):  # [C, 32, 32] -> [C, 34, 34]
        return np.pad(img, [(0, 0), (1, 1), (1, 1)])

    def wt(w_slice):  # [oc, 64ic, 3, 3] -> [ic, 9, oc]
        oc = w_slice.shape[0]
        return np.ascontiguousarray(
            w_slice.reshape(oc, 64, 9).transpose(1, 2, 0))

    in_maps = []
    for c in range(NCORES):
        b, g = c // 2, c % 2
        oc0 = g * 128
        wq_t = wt(wq[oc0:oc0 + 128])
        wk_t = wt(wk[oc0:oc0 + 128])
        wv_t = wt(wv[oc0:oc0 + 128])
        wo_t = np.ascontiguousarray(
            wo[:, oc0:oc0 + 128].reshape(64, 128, 9).transpose(1, 2, 0))
        in_maps.append({
            "qpad": pad(q[b]),
            "s01": np.concatenate([pad(s[0, b]), pad(s[1, b])], 0),
            "s23": np.concatenate([pad(s[2, b]), pad(s[3, b])], 0),
            "wq": wq_t,
            "wk": np.concatenate([wk_t, wk_t], 0),
            "wv": np.concatenate([wv_t, wv_t], 0),
            "wo": wo_t,
            "bq": bq[oc0:oc0 + 128, None],
            "bk": bk[oc0:oc0 + 128, None],
            "bv": bv[oc0:oc0 + 128, None],
            "bo": bo[:, None],
            "lnw": lnw,
            "lnb": lnb,
            "ident": ident,
        })
    return in_maps


def run(inputs, trace=False):
    """Run on the 8 cores; returns ((y, attn), BassKernelResults)."""
    _ensure_profile_hook()
    from concourse.bass_utils import run_bass_kernel_spmd

    if "nc" not in _CACHE:
        _CACHE["nc"] = _build()
    nc = _CACHE["nc"]

    in_maps = _prep_inputs(inputs)
    br = run_bass_kernel_spmd(
        nc, in_maps, core_ids=list(range(NCORES)), trace=trace)
    res = br.results

    # assemble attn: per-core attnT [4, 4096, 1024] -> attn [4, 8, 1024, 4096]
    per_b = [
        np.concatenate([res[2 * b]["attnT"], res[2 * b + 1]["attnT"]], 0)
        for b in range(B)
    ]
    attn = np.stack(per_b, 0).swapaxes(2, 3)        # view transpose
    y = np.stack([res[2 * b]["y"].reshape(64, 32, 32) for b in range(B)], 0)
    return (y, attn), br


def kernel(**inputs):
    out, _ = run(inputs, trace=False)
    return out


# revision 18
# speedup vs baseline: 1.1441x; 1.0071x over previous
"""MultiHeadImageAttentionBlock on 8 Trainium2 NeuronCores (Bass/Tile).

Sharding: core c = 2*b + g handles batch b (of 4) and head-group g (4 of 8
heads). Per core:
  - Q/K/V 3x3 convs as 9 shifted matmuls on padded 34x34 images (f32r, full
    PE rate), K/V with 2-image row-packing (images at partition 0-63/64-127).
  - Attention entirely in the transposed [k, q] layout: scores^T via PE
    (contraction d=32 at row strip 32h), exp on ACT with accum_out giving the
    softmax-over-q denominators per k for free, DVE reciprocal, DVE
    tensor_scalar per-partition multiply -> normalized attn^T tiles, which
    are both DMA'd out (host transposes the view) and fed back to the PE for
    x^T[d, q] = sum_k v^T[k, d] * attn^T[k, q] per head.
  - Partial output conv over this core's 128 channels, AllReduce over core
    pairs {2b, 2b+1}, then bias + double residual + LayerNorm -> y.

Outputs per core: attnT [4, 4096, 1024] f32 and y [64, 1024] f32.
kernel() assembles the full (y, attn) tuple matching the reference.
"""

import math
import sys
import types

import numpy as np

# ---- problem constants (hardcoded; kernel.py must be self-contained) ----
N_HEAD, D_IMAGE, D_K, D_V = 8, 64, 32, 32
H, W, KS = 32, 32, 3
N_SEQ, B = 4, 4
HW = H * W                  # 1024
NK = N_SEQ * HW             # 4096
HPG = N_HEAD // 2           # heads per group = 4
NCORES = 8
SCALE = 1.0 / math.sqrt(D_K)
LN_EPS = 1e-6

_CACHE = {}


def _ensure_profile_hook():
    """Register the axon NTFF profiling hook if the antenv shim is missing."""
    if "antenv.axon_hooks" in sys.modules:
        return
    try:
        from trn_agent_boot.trn_boot import _ntff_profile_via_ctypes
        hook = _ntff_profile_via_ctypes("/opt/axon/libaxon_pjrt.so")
    except Exception:
        hook = None
    mod = types.ModuleType("antenv.axon_hooks")
    mod.get_axon_ntff_profile_hook = lambda: hook
    mod.set_axon_ntff_profile_hook = lambda h: None
    sys.modules["antenv.axon_hooks"] = mod


def _build():
    import concourse.bacc as bacc
    import concourse.tile as tile
    from concourse import mybir

    F32 = mybir.dt.float32
    F32R = mybir.dt.float32r
    BF16 = mybir.dt.bfloat16
    FP16 = mybir.dt.float16
    EXP = mybir.ActivationFunctionType.Exp
    LNF = mybir.ActivationFunctionType.Ln
    SQF = mybir.ActivationFunctionType.Square

    nc = bacc.Bacc("TRN2", target_bir_lowering=False, debug=False,
                   num_devices=NCORES)

    # ---- DRAM I/O ----
    d_qpad = nc.dram_tensor("qpad", [64, 34, 34], F32, kind="ExternalInput")
    d_s01 = nc.dram_tensor("s01", [128, 34, 34], F32, kind="ExternalInput")
    d_s23 = nc.dram_tensor("s23", [128, 34, 34], F32, kind="ExternalInput")
    d_wq = nc.dram_tensor("wq", [64, 9, 128], F32, kind="ExternalInput")
    d_wk = nc.dram_tensor("wk", [128, 9, 128], F32, kind="ExternalInput")
    d_wv = nc.dram_tensor("wv", [128, 9, 128], F32, kind="ExternalInput")
    d_wo = nc.dram_tensor("wo", [128, 9, 64], F32, kind="ExternalInput")
    d_bq = nc.dram_tensor("bq", [128, 1], F32, kind="ExternalInput")
    d_bk = nc.dram_tensor("bk", [128, 1], F32, kind="ExternalInput")
    d_bv = nc.dram_tensor("bv", [128, 1], F32, kind="ExternalInput")
    d_bo = nc.dram_tensor("bo", [64, 1], F32, kind="ExternalInput")
    d_lnw = nc.dram_tensor("lnw", [64, 1024], F32, kind="ExternalInput")
    d_lnb = nc.dram_tensor("lnb", [64, 1024], F32, kind="ExternalInput")
    d_ident = nc.dram_tensor("ident", [128, 128], F32, kind="ExternalInput")

    d_attnT = nc.dram_tensor("attnT", [HPG, NK, HW], F32,
                             kind="ExternalOutput")
    d_y = nc.dram_tensor("y", [64, 1024], F32, kind="ExternalOutput")

    d_ccin = nc.dram_tensor("ccin", [64, 1024], F32)
    d_ccout = nc.dram_tensor("ccout", [64, 1024], F32)

    with tile.TileContext(nc) as tc:
        with (
            tc.tile_pool(name="pers", bufs=1) as pers,
            tc.tile_pool(name="stage", bufs=2) as stage,
            tc.tile_pool(name="expool", bufs=2) as expool,
            tc.tile_pool(name="attnp", bufs=2) as attnp,
            tc.tile_pool(name="expbp", bufs=2) as expbp,
            tc.tile_pool(name="small", bufs=6) as small,
            tc.tile_pool(name="ps", bufs=3, space="PSUM") as ps,
            tc.tile_pool(name="xps", bufs=1, space="PSUM") as xps,
        ):
            # ---- load + round inputs ----
            def load_round(dram, shape, rdtype):
                st = stage.tile(shape, F32, tag="ldstage", name="st")
                nc.sync.dma_start(st[:], dram.ap()[:])
                rt = pers.tile(shape, rdtype, tag=dram.name + "_r",
                               name=dram.name + "_r")
                nc.vector.tensor_copy(rt[:], st[:])
                return rt

            qpad_f = pers.tile([64, 34, 34], F32, tag="qpad_f")
            nc.sync.dma_start(qpad_f[:], d_qpad.ap()[:])
            qpad = pers.tile([64, 34, 34], FP16, tag="qpad_r")
            nc.vector.tensor_copy(qpad[:], qpad_f[:])
            s01 = load_round(d_s01, [128, 34, 34], FP16)
            s23 = load_round(d_s23, [128, 34, 34], FP16)
            wq = load_round(d_wq, [64, 9, 128], FP16)
            wk = load_round(d_wk, [128, 9, 128], FP16)
            wv = load_round(d_wv, [128, 9, 128], FP16)
            ident_r = load_round(d_ident, [128, 128], F32R)

            def load_f32(dram, shape, tag):
                t = pers.tile(shape, F32, tag=tag, name=tag)
                nc.sync.dma_start(t[:], dram.ap()[:])
                return t

            wo = load_f32(d_wo, [128, 9, 64], "wo")
            bq = load_f32(d_bq, [128, 1], "bq")
            bk = load_f32(d_bk, [128, 1], "bk")
            bv = load_f32(d_bv, [128, 1], "bv")
            bo = load_f32(d_bo, [64, 1], "bo")
            lnw = load_f32(d_lnw, [64, 1024], "lnw")
            lnb = load_f32(d_lnb, [64, 1024], "lnb")

            # ---- conv: 3x3 SAME as 9 shifted matmuls ----
            def conv_mms(psum_t, w_t, img_t, ic0, pos):
                for o in range(9):
                    ky, kx = o // 3, o % 3
                    for rh in range(2):
                        nc.tensor.matmul(
                            psum_t[:, rh * 512:(rh + 1) * 512],
                            w_t[ic0:ic0 + 64, o, :],
                            img_t[ic0:ic0 + 64,
                                  ky + rh * 16: ky + rh * 16 + 16,
                                  kx: kx + 32],
                            start=(o == 0), stop=(o == 8),
                            tile_position=pos,
                        )

            # Q conv -> qconv [128, 1024] f32r
            qps = ps.tile([128, 1024], F32, tag="ps1")
            conv_mms(qps, wq, qpad, 0, (0, 0))
            qconv = pers.tile([128, 1024], FP16, tag="qconv")
            nc.vector.tensor_scalar_add(qconv[:], qps[:], bq[:])

            # K/V convs over 4 seq images (2-image row packing,
            # halves interleaved so the two row strips run concurrently)
            kconv = pers.tile([128, NK], FP16, tag="kconv")
            vconv = pers.tile([128, NK], F32R, tag="vconv")

            def conv_pair(w_t, img_t, p0, p1):
                for o in range(9):
                    ky, kx = o // 3, o % 3
                    for rh in range(2):
                        for half, pt in ((0, p0), (1, p1)):
                            nc.tensor.matmul(
                                pt[:, rh * 512:(rh + 1) * 512],
                                w_t[64 * half:64 * half + 64, o, :],
                                img_t[64 * half:64 * half + 64,
                                      ky + rh * 16: ky + rh * 16 + 16,
                                      kx: kx + 32],
                                start=(o == 0), stop=(o == 8),
                                tile_position=(64 * half, 0),
                            )

            def conv_pair_imgs(pair, img_t):
                kp0 = ps.tile([128, 1024], F32, tag="ps1", name="kp0")
                kp1 = ps.tile([128, 1024], F32, tag="ps1", name="kp1")
                conv_pair(wk, img_t, kp0, kp1)
                for half, pt in ((0, kp0), (1, kp1)):
                    img = 2 * pair + half
                    nc.vector.tensor_scalar_add(
                        kconv[:, img * 1024:(img + 1) * 1024], pt[:], bk[:])
                vp0 = ps.tile([128, 1024], F32, tag="ps1", name="vp0")
                vp1 = ps.tile([128, 1024], F32, tag="ps1", name="vp1")
                conv_pair(wv, img_t, vp0, vp1)
                for half, pt in ((0, vp0), (1, vp1)):
                    img = 2 * pair + half
                    nc.vector.tensor_scalar_add(
                        vconv[:, img * 1024:(img + 1) * 1024], pt[:], bv[:])

            def vtrans(kc):
                vtp = ps.tile([128, 128], F32R, tag="ps1", name="vtp")
                nc.tensor.transpose(
                    vtp[:], vconv[:, kc * 128:(kc + 1) * 128], ident_r[:])
                nc.vector.tensor_copy(vT[:, kc, :], vtp[:])

            vT = pers.tile([128, 32, 128], BF16, tag="vT")
            xpad = pers.tile([128, 34, 34], F32, tag="xpad")
            nc.vector.memset(xpad[:], 0.0)
            xT = xps.tile([128, 1024], F32, tag="xT", name="xT")

            def main_kc(kc):
                att_st = attnp.tile([128, HPG, 1024], F32R, tag="att",
                                    name="att_st")
                for h in range(HPG):
                    hp = 32 * h
                    sTh = ps.tile([128, 1024], F32, tag="ps1", name="sTh")
                    for qh in range(2):
                        nc.tensor.matmul(
                            sTh[:, qh * 512:(qh + 1) * 512],
                            kconv[hp:hp + 32, kc * 128:(kc + 1) * 128],
                            qconv[hp:hp + 32, qh * 512:(qh + 1) * 512],
                            start=True, stop=True,
                            tile_position=(hp, 0),
                        )
                    expT = expool.tile([128, 1024], F32, tag="expT",
                                       name="expT")
                    cs = small.tile([128, 1], F32, tag="cs", name="cs")
                    nc.scalar.activation(expT[:], sTh[:], EXP,
                                         scale=SCALE, accum_out=cs[:])
                    rs = small.tile([128, 1], F32, tag="rs", name="rs")
                    nc.vector.reciprocal(rs[:], cs[:])
                    nc.vector.tensor_scalar_mul(
                        att_st[:, h, :], expT[:], rs[:])
                    expb = expbp.tile([128, 1024], BF16, tag="expb",
                                      name="expb")
                    nc.vector.tensor_copy(expb[:], att_st[:, h, :])
                    for qh in range(2):
                        nc.tensor.matmul(
                            xT[hp:hp + 32, qh * 512:(qh + 1) * 512],
                            vT[:, kc, hp:hp + 32],
                            expb[:, qh * 512:(qh + 1) * 512],
                            start=(kc == 0), stop=(kc == 31),
                            tile_position=(0, hp),
                        )
                out_ap = d_attnT.ap()[
                    :, kc * 128:(kc + 1) * 128, :
                ].rearrange("h p q -> p h q")
                nc.sync.dma_start(out_ap, att_st[:].bitcast(F32))

            for pair, img_t in ((0, s01), (1, s23)):
                conv_pair_imgs(pair, img_t)
            for kc in range(32):
                vtrans(kc)
            for kc in range(32):
                main_kc(kc)
            nc.vector.tensor_copy(xpad[:, 1:33, 1:33], xT[:])


            # ---- output conv (partial over this core's 128 channels) ----
            ops_t = ps.tile([128, 1024], F32, tag="ps1", name="ops_t")
            for o in range(9):
                ky, kx = o // 3, o % 3
                for rh in range(2):
                    nc.tensor.matmul(
                        ops_t[0:64, rh * 512:(rh + 1) * 512],
                        wo[:, o, :],
                        xpad[:, ky + rh * 16: ky + rh * 16 + 16, kx: kx + 32],
                        start=(o == 0), stop=(o == 8),
                    )
            osb = pers.tile([64, 1024], F32, tag="osb")
            nc.vector.tensor_copy(osb[:], ops_t[0:64, :])
            nc.sync.dma_start(d_ccin.ap()[:], osb[:])
            nc.gpsimd.collective_compute(
                "AllReduce", mybir.AluOpType.add,
                replica_groups=[[0, 1], [2, 3], [4, 5], [6, 7]],
                ins=[d_ccin.ap()[:]],
                outs=[d_ccout.ap()[:]],
            )
            ofull = pers.tile([64, 1024], F32, tag="ofull")
            nc.sync.dma_start(ofull[:], d_ccout.ap()[:])

            # t = ofull + bo + 2*query
            t_t = pers.tile([64, 1024], F32, tag="t_t")
            nc.vector.tensor_scalar_add(t_t[:], ofull[:], bo[:])
            q2 = pers.tile([64, 1024], F32, tag="q2")
            nc.vector.tensor_scalar_mul(q2[:], qpad_f[:, 1:33, 1:33], 2.0)
            nc.vector.tensor_tensor(
                out=t_t[:], in0=t_t[:], in1=q2[:], op=mybir.AluOpType.add)

            # stats over all 64*1024 elements
            stats = pers.tile([64, 2], F32, tag="stats")
            nc.vector.tensor_reduce(
                stats[:, 0:1], t_t[:], axis=mybir.AxisListType.X,
                op=mybir.AluOpType.add)
            junk = pers.tile([64, 1024], F32, tag="junk")
            nc.scalar.activation(junk[:], t_t[:], SQF,
                                 accum_out=stats[:, 1:2])
            ones64 = pers.tile([64, 1], F32, tag="ones64")
            nc.vector.memset(ones64[:], 1.0)
            onesr = pers.tile([1, 64], F32, tag="onesr")
            nc.vector.memset(onesr[:], 1.0)
            stp = ps.tile([128, 1024], F32, tag="ps1", name="stp")
            nc.tensor.matmul(stp[0:1, 0:2], ones64[:], stats[:],
                             start=True, stop=True)
            mr = pers.tile([1, 2], F32, tag="mr")     # [mean, rstd]
            nc.vector.tensor_scalar_mul(mr[:, 0:1], stp[0:1, 0:1],
                                        1.0 / 65536.0)
            et2 = pers.tile([1, 1], F32, tag="et2")
            nc.vector.tensor_scalar_mul(et2[:], stp[0:1, 1:2], 1.0 / 65536.0)
            m2 = pers.tile([1, 1], F32, tag="m2")
            nc.vector.tensor_tensor(out=m2[:], in0=mr[:, 0:1], in1=mr[:, 0:1],
                                    op=mybir.AluOpType.mult)
            var = pers.tile([1, 1], F32, tag="var")
            nc.vector.tensor_tensor(out=var[:], in0=et2[:], in1=m2[:],
                                    op=mybir.AluOpType.subtract)
            lnv = pers.tile([1, 1], F32, tag="lnv")
            epst = pers.tile([1, 1], F32, tag="epst")
            nc.vector.memset(epst[:], LN_EPS)
            nc.scalar.activation(lnv[:], var[:], LNF, bias=epst[:])
            nc.scalar.activation(mr[:, 1:2], lnv[:], EXP, scale=-0.5)
            bcp = ps.tile([128, 1024], F32, tag="ps1", name="bcp")
            nc.tensor.matmul(bcp[0:64, 0:2], onesr[:, :], mr[:],
                             start=True, stop=True)
            bc = pers.tile([64, 2], F32, tag="bc")
            nc.vector.tensor_copy(bc[:], bcp[0:64, 0:2])
            # y = ((t - mean) * rstd) * lnw + lnb
            u = pers.tile([64, 1024], F32, tag="u")
            nc.vector.tensor_scalar(
                out=u[:], in0=t_t[:],
                scalar1=bc[:, 0:1], scalar2=bc[:, 1:2],
                op0=mybir.AluOpType.subtract, op1=mybir.AluOpType.mult)
            nc.vector.tensor_tensor(out=u[:], in0=u[:], in1=lnw[:],
                                    op=mybir.AluOpType.mult)
            nc.vector.tensor_tensor(out=u[:], in0=u[:], in1=lnb[:],
                                    op=mybir.AluOpType.add)
            nc.sync.dma_start(d_y.ap()[:], u[:])

    nc.compile()
    return nc


def _prep_inputs(inputs):
    """Host-side sharding/layout prep (data movement only)."""
    q = np.asarray(inputs["query_image"], np.float32)
    s = np.asarray(inputs["seq_images"], np.float32)
    wq = np.asarray(inputs["wq"], np.float32)
    bq = np.asarray(inputs["bq"], np.float32)
    wk = np.asarray(inputs["wk"], np.float32)
    bk = np.asarray(inputs["bk"], np.float32)
    wv = np.asarray(inputs["wv"], np.float32)
    bv = np.asarray(inputs["bv"], np.float32)
    wo = np.asarray(inputs["wo"], np.float32)
    bo = np.asarray(inputs["bo"], np.float32)
    lnw = np.asarray(inputs["ln_w"], np.float32).reshape(64, 1024)
    lnb = np.asarray(inputs["ln_b"], np.float32).reshape(64, 1024)
    ident = np.eye(128, dtype=np.float32)

    def pad(img):  # [C, 32, 32] -> [C, 34, 34]
        return np.pad(img, [(0, 0), (1, 1), (1, 1)])

    def wt(w_slice):  # [oc, 64ic, 3, 3] -> [ic, 9, oc]
        oc = w_slice.shape[0]
        return np.ascontiguousarray(
            w_slice.reshape(oc, 64, 9).transpose(1, 2, 0))

    in_maps = []
    for c in range(NCORES):
        b, g = c // 2, c % 2
        oc0 = g * 128
        wq_t = wt(wq[oc0:oc0 + 128])
        wk_t = wt(wk[oc0:oc0 + 128])
        wv_t = wt(wv[oc0:oc0 + 128])
        wo_t = np.ascontiguousarray(
            wo[:, oc0:oc0 + 128].reshape(64, 128, 9).transpose(1, 2, 0))
        in_maps.append({
            "qpad": pad(q[b]),
            "s01": np.concatenate([pad(s[0, b]), pad(s[1, b])], 0),
            "s23": np.concatenate([pad(s[2, b]), pad(s[3, b])], 0),
            "wq": wq_t,
            "wk": np.concatenate([wk_t, wk_t], 0),
            "wv": np.concatenate([wv_t, wv_t], 0),
            "wo": wo_t,
            "bq": bq[oc0:oc0 + 128, None],
            "bk": bk[oc0:oc0 + 128, None],
            "bv": bv[oc0:oc0 + 128, None],
            "bo": bo[:, None],
            "lnw": lnw,
            "lnb": lnb,
            "ident": ident,
        })
    return in_maps


def run(inputs, trace=False):
    """Run on the 8 cores; returns ((y, attn), BassKernelResults)."""
    _ensure_profile_hook()
    from concourse.bass_utils import run_bass_kernel_spmd

    if "nc" not in _CACHE:
        _CACHE["nc"] = _build()
    nc = _CACHE["nc"]

    in_maps = _prep_inputs(inputs)
    br = run_bass_kernel_spmd(
        nc, in_maps, core_ids=list(range(NCORES)), trace=trace)
    res = br.results

    # assemble attn: per-core attnT [4, 4096, 1024] -> attn [4, 8, 1024, 4096]
    per_b = [
        np.concatenate([res[2 * b]["attnT"], res[2 * b + 1]["attnT"]], 0)
        for b in range(B)
    ]
    attn = np.stack(per_b, 0).swapaxes(2, 3)        # view transpose
    y = np.stack([res[2 * b]["y"].reshape(64, 32, 32) for b in range(B)], 0)
    return (y, attn), br


def kernel(**inputs):
    out, _ = run(inputs, trace=False)
    return out
